# revision 57
# baseline (speedup 1.0000x reference)
"""GQA kernel for 8 TRN2 NeuronCores.

Model: B=4, T=2048, C=1024, 16 q heads / 4 kv heads / head_dim 64, causal.
Sharding: 16 (batch, kv-head) units -> 2 per core. Core c handles batch c//2
and kv-head pair (0,1) if c even else (2,3), i.e. q heads 0-7 or 8-15.
Each core computes its slice of the Q/K/V projections, local causal
attention, and a partial output projection (its 512 columns of the head
concat); the host sums the two partial y's per batch.

All matmul operands are bf16 (1 cycle/row on the PE vs 4 for fp32);
accumulation stays fp32 in PSUM. The kernel is a single software pipeline
over token blocks t: project Q/K/V for block t, then attention for query
block c=t (which only needs K/V up to block t), with the output projection
and softmax-normalize tails deferred one step so their serial chains hide
under the next block's dense matmul work. Scores for a head pair land in
one [128, 2, 512] PSUM tile (A/B halves) so one Exp activation covers both,
and causal-diagonal blocks only compute the unmasked column suffix.

Device layouts (host pre-transposes and pre-casts to bf16):
  xT  [C, T]    x[b].T
  wqT [C, 512]  Wq rows for local heads, pair-permuted, transposed
  wkT [C, 128]  Wk rows for the 2 kv heads, transposed
  wvT [C, 128]
  woT [512, C]  Wo cols for local heads, pair-permuted, transposed
Head pair p = (local head p of kvA, local head p of kvB) shares one qT tile
([128, T]: rows 0-63 head A, 64-127 head B) so scores run as two row-tiled
K=64 matmuls into the same PSUM tile. PV uses Vext=[V|1] (M=65) so the
softmax denominator falls out of row 64 of the PV accumulator.
"""

import numpy as np

T = 2048
C = 1024
HD = 64
P = 128
TQ = 512
NTQ = T // TQ  # 4
NTK = T // P   # 16
QCOLS = 512

_PROG = None


def _build_program():
    import concourse.mybir as mybir
    import concourse.tile as tile
    from concourse import bacc

    FP32 = mybir.dt.float32
    BF16 = mybir.dt.bfloat16
    AF = mybir.ActivationFunctionType
    ALU = mybir.AluOpType

    nc = bacc.Bacc("TRN2", target_bir_lowering=False, debug=False, num_devices=8)

    xT = nc.dram_tensor("xT", [C, T], BF16, kind="ExternalInput").ap()
    wqT = nc.dram_tensor("wqT", [C, QCOLS], BF16, kind="ExternalInput").ap()
    wkT = nc.dram_tensor("wkT", [C, 128], BF16, kind="ExternalInput").ap()
    wvT = nc.dram_tensor("wvT", [C, 128], BF16, kind="ExternalInput").ap()
    woT = nc.dram_tensor("woT", [QCOLS, C], BF16, kind="ExternalInput").ap()
    y = nc.dram_tensor("y", [T, C], FP32, kind="ExternalOutput").ap()

    with tile.TileContext(nc) as tc:
        with tc.tile_pool(name="const", bufs=1) as cpool, \
             tc.tile_pool(name="persist", bufs=1) as pp, \
             tc.tile_pool(name="xw", bufs=1) as xw, \
             tc.tile_pool(name="ptsb", bufs=6) as ptsb, \
             tc.tile_pool(name="nrm", bufs=3) as nrm, \
             tc.tile_pool(name="ysb", bufs=4) as ysb, \
             tc.tile_pool(name="scps", bufs=2, space="PSUM") as scps, \
             tc.tile_pool(name="otps", bufs=2, space="PSUM") as otps, \
             tc.tile_pool(name="pyps", bufs=2, space="PSUM") as pyps:
            # ident2: two stacked 64x64 identity blocks, so a transpose whose
            # input lives at base partition 64 can use ident2[64:128, :]
            ident2 = cpool.tile([P, 64], BF16, tag="ident2")
            nc.gpsimd.memset(ident2[:], 0.0)
            for blk in range(2):
                nc.gpsimd.affine_select(
                    out=ident2[:], in_=ident2[:],
                    compare_op=ALU.not_equal, fill=1.0,
                    base=-64 * blk, pattern=[[-1, 64]], channel_multiplier=1,
                )
            # K=65 denominator-broadcast selector: row 0 -> partitions 0-63,
            # row 64 -> partitions 64-127 (rows 1-63 are zero, matching the
            # always-zero middle rows of the dd tile below)
            ones2 = cpool.tile([HD + 1, P], BF16, tag="ones2")
            nc.vector.memset(ones2[:], 0.0)
            nc.vector.memset(ones2[0:1, 0:64], 1.0)
            nc.vector.memset(ones2[64:65, 64:128], 1.0)

            qTs = [pp.tile([P, T], BF16, tag=f"qT{i}", name=f"qT{i}") for i in range(4)]
            kT = pp.tile([P, T], BF16, tag="kT")
            vT = pp.tile([P, T], BF16, tag="vT")
            # vext[kv * NTK + j] = [V_kv tile j | ones]  [128, 65]
            vext = [pp.tile([P, HD + 1], BF16, tag=f"ve{i}", name=f"ve{i}") for i in range(2 * NTK)]
            attnT = [pp.tile([P, T], BF16, tag=f"at{i}", name=f"at{i}") for i in range(4)]
            # dd[0] / dd[64] hold the pair's raw softmax denominators; the
            # middle rows stay zero (multiplied by ones2's zero rows)
            dd = pp.tile([HD + 1, TQ], BF16, tag="dd")
            nc.vector.memset(dd[:], 0.0)

            xts = [xw.tile([P, T], BF16, tag=f"x{j}", name=f"x{j}") for j in range(8)]
            wq = [xw.tile([P, QCOLS], BF16, tag=f"wq{j}", name=f"wq{j}") for j in range(8)]
            wk = [xw.tile([P, 128], BF16, tag=f"wk{j}", name=f"wk{j}") for j in range(8)]
            wv = [xw.tile([P, 128], BF16, tag=f"wv{j}", name=f"wv{j}") for j in range(8)]
            wo = [xw.tile([P, C], BF16, tag=f"wo{p}", name=f"wo{p}") for p in range(4)]

            # wq + the first x column-chunk first so Q-proj t=0 can start
            # early; alternate between the two HWDGE queues (SP / ACT) so the
            # ramp isn't serialized on one issue engine. ACT itself is idle
            # until attention starts, so borrowing its queue is free here.
            def dma_in(i, out, in_):
                eng = nc.sync if i % 2 == 0 else nc.scalar
                eng.dma_start(out=out, in_=in_)

            for j in range(8):
                dma_in(j, wq[j][:], wqT[P * j:P * (j + 1), :])
                dma_in(j + 1, xts[j][:, 0:TQ], xT[P * j:P * (j + 1), 0:TQ])
            for j in range(8):
                dma_in(j, wk[j][:], wkT[P * j:P * (j + 1), :])
                dma_in(j + 1, wv[j][:], wvT[P * j:P * (j + 1), :])
            for t in range(1, NTQ):
                for j in range(8):
                    # later chunks stay on the SP queue: the ACT queue must be
                    # clear before the first exps start
                    eng = nc.scalar if t == 1 and j % 2 == 0 else nc.sync
                    eng.dma_start(
                        out=xts[j][:, TQ * t:TQ * (t + 1)],
                        in_=xT[P * j:P * (j + 1), TQ * t:TQ * (t + 1)],
                    )
            for p in range(4):
                dma_in(p, wo[p][:], woT[P * p:P * (p + 1), :])

            # Deferred-emission queue: normalize tails and out-proj blocks
            # are emitted one pair-loop later so their serial DVE/PE chains
            # overlap the next pair's dense matmul work instead of
            # head-of-line blocking the PE.
            pending = []

            def norm_tail(p, cs, oA, oB, last=False):
                def emit():
                    # stack both raw denominator rows into dd, broadcast A
                    # onto partitions 0-63 / B onto 64-127 with one K=65
                    # matmul, reciprocal on all 128 lanes, then scale the PV
                    # outputs into attnT (all-bf16 muls hit the DVE fast path).
                    # For the last pair, oA/oB are the PSUM accumulators
                    # themselves (no staging copies — the ot slots are not
                    # needed again, and bc comes from the py tag to avoid
                    # deadlocking on the held ot slots).
                    nc.vector.tensor_copy(dd[0:1, :], oA[64:65, :])
                    nc.vector.tensor_copy(dd[64:65, :], oB[64:65, :])
                    bcpool = pyps if last else otps
                    bc = bcpool.tile([P, TQ], FP32, tag="py" if last else "ot",
                                     name="bc")
                    nc.tensor.matmul(bc[:], ones2[:], dd[:], start=True, stop=True)
                    rec = nrm.tile([P, TQ], FP32, tag="rec", name="rec")
                    nc.vector.reciprocal_approx_fast(rec[:], bc[:])
                    if last:
                        # PSUM operand + one fp32 SBUF operand: no bf16
                        # staging needed, and the SBUF same-start-partition
                        # rule sees only one SBUF input per mul
                        nc.vector.tensor_mul(attnT[p][0:64, cs], oA[0:64, :], rec[0:64, :])
                        nc.vector.tensor_mul(attnT[p][64:128, cs], oB[0:64, :], rec[64:128, :])
                        return
                    # split to two partition-0-based tiles: TensorTensor needs
                    # both SBUF inputs at the same start partition (copies may
                    # shift partitions, 2-input ALU ops may not)
                    recbA = nrm.tile([64, TQ], BF16, tag="recbA", name="recbA")
                    recbB = nrm.tile([64, TQ], BF16, tag="recbB", name="recbB")
                    nc.vector.tensor_copy(recbA[:], rec[0:64, :])
                    nc.vector.tensor_copy(recbB[:], rec[64:128, :])
                    nc.vector.tensor_mul(attnT[p][0:64, cs], oA[0:64, :], recbA[:])
                    nc.vector.tensor_mul(attnT[p][64:128, cs], oB[0:64, :], recbB[:])
                return emit

            def out_proj(c):
                def emit():
                    for tt in range(4):
                        t = 4 * c + tt
                        for co in range(2):
                            ps = pyps.tile([P, TQ], FP32, tag="py", name="yps")
                            for p in range(4):
                                nc.tensor.matmul(
                                    ps[:],
                                    attnT[p][:, P * t:P * (t + 1)],
                                    wo[p][:, TQ * co:TQ * (co + 1)],
                                    start=(p == 0), stop=(p == 3),
                                )
                            yt = ysb.tile([P, TQ], FP32, tag="yt")
                            nc.vector.tensor_copy(yt[:], ps[:])
                            nc.sync.dma_start(
                                out=y[P * t:P * (t + 1), TQ * co:TQ * (co + 1)],
                                in_=yt[:],
                            )
                return emit

            for t in range(NTQ):
                ts = slice(TQ * t, TQ * (t + 1))
                # ---- projections for token block t
                for f in range(4):
                    ps = pyps.tile([P, TQ], FP32, tag="py", name="qps")
                    for k in range(8):
                        nc.tensor.matmul(
                            ps[:], wq[k][:, P * f:P * (f + 1)], xts[k][:, ts],
                            start=(k == 0), stop=(k == 7),
                        )
                    nc.vector.tensor_copy(qTs[f][:, ts], ps[:])
                ps = pyps.tile([P, TQ], FP32, tag="py", name="kps")
                for k in range(8):
                    nc.tensor.matmul(
                        ps[:], wk[k][:], xts[k][:, ts],
                        start=(k == 0), stop=(k == 7),
                    )
                nc.vector.tensor_copy(kT[:, ts], ps[:])
                ps = pyps.tile([P, TQ], FP32, tag="py", name="vps")
                for k in range(8):
                    nc.tensor.matmul(
                        ps[:], wv[k][:], xts[k][:, ts],
                        start=(k == 0), stop=(k == 7),
                    )
                nc.vector.tensor_copy(vT[:, ts], ps[:])
                # V to token-major via PE transpose: [64, 128] -> [128, 64]
                for kv in range(2):
                    for jj in range(4):
                        j = 4 * t + jj
                        tp = pyps.tile([P, HD], BF16, tag="py", name="vtp")
                        nc.tensor.transpose(
                            tp[:],
                            vT[64 * kv:64 * kv + 64, P * j:P * (j + 1)],
                            ident2[64 * kv:64 * kv + 64, :],
                        )
                        ve = vext[kv * NTK + j]
                        nc.vector.tensor_copy(ve[:, 0:HD], tp[:])
                        nc.gpsimd.memset(ve[:, HD:HD + 1], 1.0)

                # ---- attention for query block c = t
                c = t
                cs = ts
                for p in range(4):
                    if c == NTQ - 1 and p == 3:
                        # flush queued normalize chains (DVE-bound, no PE
                        # cost) before the final pair's j-loop so they hide
                        # under its dense matmul work; out-proj thunks stay
                        # queued — their PE work fills this j-loop's
                        # ACT-gated bubbles
                        keep = []
                        for kind, thunk in pending:
                            if kind == "norm":
                                thunk()
                            else:
                                keep.append((kind, thunk))
                        pending[:] = keep
                    outA = otps.tile([HD + 1, TQ], FP32, tag="ot", name="otA")
                    outB = otps.tile([HD + 1, TQ], FP32, tag="ot", name="otB")
                    jmax = 4 * c + 3
                    for j in range(jmax + 1):
                        # scores for the pair land in one 2-bank tile:
                        # [:, 0, :] head A, [:, 1, :] head B
                        s = scps.tile([P, 2, TQ], FP32, tag="s")
                        nc.tensor.matmul(
                            s[:, 0, :],
                            kT[0:64, P * j:P * (j + 1)],
                            qTs[p][0:64, cs],
                            start=True, stop=True, tile_position=(0, 0),
                        )
                        nc.tensor.matmul(
                            s[:, 1, :],
                            kT[64:128, P * j:P * (j + 1)],
                            qTs[p][64:128, cs],
                            start=True, stop=True, tile_position=(64, 0),
                        )
                        pt = ptsb.tile([P, 2, TQ], BF16, tag="pt", name="pt")
                        r = j - 4 * c
                        # diagonal blocks (r >= 0): columns below 128r are
                        # fully masked — skip exp/mask/PV on them entirely
                        lo = 128 * r if r > 0 else 0
                        nc.scalar.activation(
                            pt[:, :, lo:TQ], s[:, :, lo:TQ], AF.Exp, scale=0.125
                        )
                        if r >= 0:
                            # only the 128-wide boundary block straddles the
                            # causal edge: keep where x >= tk in it
                            for h in range(2):
                                nc.gpsimd.affine_select(
                                    out=pt[:, h, lo:lo + P],
                                    in_=pt[:, h, lo:lo + P],
                                    compare_op=ALU.is_ge,
                                    fill=0.0,
                                    base=0,
                                    pattern=[[1, P]],
                                    channel_multiplier=-1,
                                )
                        nc.tensor.matmul(
                            outA[:, lo:TQ], vext[j][:], pt[:, 0, lo:TQ],
                            start=(j == 0), stop=(j == jmax),
                        )
                        nc.tensor.matmul(
                            outB[:, lo:TQ], vext[NTK + j][:], pt[:, 1, lo:TQ],
                            start=(j == 0), stop=(j == jmax),
                        )

                    # PSUM -> SBUF copies emitted immediately so the ot slots
                    # recycle fast; the rest of the normalize is deferred one
                    # pair-loop
                    if c == NTQ - 1 and p == 3:
                        # last pair: normalize straight from the PSUM
                        # accumulators, inline — nothing left to hide behind
                        norm_tail(p, cs, outA, outB, last=True)()
                    else:
                        oA = nrm.tile([HD + 1, TQ], BF16, tag="oA", name="oA")
                        oB = nrm.tile([HD + 1, TQ], BF16, tag="oB", name="oB")
                        nc.vector.tensor_copy(oA[:], outA[:])
                        nc.vector.tensor_copy(oB[:], outB[:])
                        npop = 2 if len(pending) >= 3 else 1
                        for _ in range(min(npop, len(pending))):
                            pending.pop(0)[1]()
                        pending.append(("norm", norm_tail(p, cs, oA, oB)))
                pending.append(("proj", out_proj(c)))
            for _, thunk in pending:
                thunk()

    nc.compile()
    return nc


def get_program():
    global _PROG
    if _PROG is None:
        _PROG = _build_program()
    return _PROG


def make_in_maps(x, Wq, Wk, Wv, Wo):
    """Build the per-core input dicts (host-side sharding + layout prep)."""
    import ml_dtypes
    BF = ml_dtypes.bfloat16
    x = np.asarray(x, np.float32)
    Wq = np.asarray(Wq, np.float32)
    Wk = np.asarray(Wk, np.float32)
    Wv = np.asarray(Wv, np.float32)
    Wo = np.asarray(Wo, np.float32)
    in_maps = []
    for core in range(8):
        b, half = core // 2, core % 2
        h0 = 8 * half
        kv0 = 2 * half
        # pair-permuted local head order: [h0, h0+4, h0+1, h0+5, ...]
        heads = []
        for p in range(4):
            heads += [h0 + p, h0 + p + 4]
        qrows = np.concatenate([Wq[h * HD:(h + 1) * HD] for h in heads], 0)  # [512, C]
        wocols = np.concatenate([Wo[:, h * HD:(h + 1) * HD] for h in heads], 1)  # [C, 512]
        in_maps.append({
            "xT": np.ascontiguousarray(x[b].T.astype(BF)),
            "wqT": np.ascontiguousarray(qrows.T.astype(BF)),
            "wkT": np.ascontiguousarray(Wk[kv0 * HD:(kv0 + 2) * HD].T.astype(BF)),
            "wvT": np.ascontiguousarray(Wv[kv0 * HD:(kv0 + 2) * HD].T.astype(BF)),
            "woT": np.ascontiguousarray(wocols.T.astype(BF)),
        })
    return in_maps


def run_on_hw(in_maps, trace=False, **kw):
    from concourse.bass_utils import run_bass_kernel_spmd
    nc = get_program()
    return run_bass_kernel_spmd(nc, in_maps, list(range(8)), trace=trace, **kw)


def kernel(**inputs):
    in_maps = make_in_maps(
        inputs["x"], inputs["Wq"], inputs["Wk"], inputs["Wv"], inputs["Wo"]
    )
    res = run_on_hw(in_maps)
    out = np.empty((4, T, C), np.float32)
    for b in range(4):
        out[b] = res.results[2 * b]["y"] + res.results[2 * b + 1]["y"]
    return out


# revision 63
# speedup vs baseline: 1.0019x; 1.0019x over previous
"""GQA kernel for 8 TRN2 NeuronCores.

Model: B=4, T=2048, C=1024, 16 q heads / 4 kv heads / head_dim 64, causal.
Sharding: 16 (batch, kv-head) units -> 2 per core. Core c handles batch c//2
and kv-head pair (0,1) if c even else (2,3), i.e. q heads 0-7 or 8-15.
Each core computes its slice of the Q/K/V projections, local causal
attention, and a partial output projection (its 512 columns of the head
concat); the host sums the two partial y's per batch.

All matmul operands are bf16 (1 cycle/row on the PE vs 4 for fp32);
accumulation stays fp32 in PSUM. The kernel is a single software pipeline
over token blocks t: project Q/K/V for block t, then attention for query
block c=t (which only needs K/V up to block t), with the output projection
and softmax-normalize tails deferred one step so their serial chains hide
under the next block's dense matmul work. Scores for a head pair land in
one [128, 2, 512] PSUM tile (A/B halves) so one Exp activation covers both,
and causal-diagonal blocks only compute the unmasked column suffix.

Device layouts (host pre-transposes and pre-casts to bf16):
  xT  [C, T]    x[b].T
  wqT [C, 512]  Wq rows for local heads, pair-permuted, transposed
  wkT [C, 128]  Wk rows for the 2 kv heads, transposed
  wvT [C, 128]
  woT [512, C]  Wo cols for local heads, pair-permuted, transposed
Head pair p = (local head p of kvA, local head p of kvB) shares one qT tile
([128, T]: rows 0-63 head A, 64-127 head B) so scores run as two row-tiled
K=64 matmuls into the same PSUM tile. PV uses Vext=[V|1] (M=65) so the
softmax denominator falls out of row 64 of the PV accumulator.
"""

import numpy as np

T = 2048
C = 1024
HD = 64
P = 128
TQ = 512
NTQ = T // TQ  # 4
NTK = T // P   # 16
QCOLS = 512

_PROG = None


def _build_program():
    import concourse.mybir as mybir
    import concourse.tile as tile
    from concourse import bacc

    FP32 = mybir.dt.float32
    BF16 = mybir.dt.bfloat16
    AF = mybir.ActivationFunctionType
    ALU = mybir.AluOpType

    nc = bacc.Bacc("TRN2", target_bir_lowering=False, debug=False, num_devices=8)

    xT = nc.dram_tensor("xT", [C, T], BF16, kind="ExternalInput").ap()
    wqT = nc.dram_tensor("wqT", [C, QCOLS], BF16, kind="ExternalInput").ap()
    wkT = nc.dram_tensor("wkT", [C, 128], BF16, kind="ExternalInput").ap()
    wvT = nc.dram_tensor("wvT", [C, 128], BF16, kind="ExternalInput").ap()
    woT = nc.dram_tensor("woT", [QCOLS, C], BF16, kind="ExternalInput").ap()
    y = nc.dram_tensor("y", [T, C], FP32, kind="ExternalOutput").ap()

    with tile.TileContext(nc) as tc:
        with tc.tile_pool(name="const", bufs=1) as cpool, \
             tc.tile_pool(name="persist", bufs=1) as pp, \
             tc.tile_pool(name="xw", bufs=1) as xw, \
             tc.tile_pool(name="ptsb", bufs=6) as ptsb, \
             tc.tile_pool(name="nrm", bufs=3) as nrm, \
             tc.tile_pool(name="ysb", bufs=4) as ysb, \
             tc.tile_pool(name="scps", bufs=2, space="PSUM") as scps, \
             tc.tile_pool(name="otps", bufs=2, space="PSUM") as otps, \
             tc.tile_pool(name="pyps", bufs=2, space="PSUM") as pyps:
            # ident2: two stacked 64x64 identity blocks, so a transpose whose
            # input lives at base partition 64 can use ident2[64:128, :]
            ident2 = cpool.tile([P, 64], BF16, tag="ident2")
            nc.gpsimd.memset(ident2[:], 0.0)
            for blk in range(2):
                nc.gpsimd.affine_select(
                    out=ident2[:], in_=ident2[:],
                    compare_op=ALU.not_equal, fill=1.0,
                    base=-64 * blk, pattern=[[-1, 64]], channel_multiplier=1,
                )
            # K=65 denominator-broadcast selector: row 0 -> partitions 0-63,
            # row 64 -> partitions 64-127 (rows 1-63 are zero, matching the
            # always-zero middle rows of the dd tile below)
            ones2 = cpool.tile([HD + 1, P], BF16, tag="ones2")
            nc.vector.memset(ones2[:], 0.0)
            nc.vector.memset(ones2[0:1, 0:64], 1.0)
            nc.vector.memset(ones2[64:65, 64:128], 1.0)

            qTs = [pp.tile([P, T], BF16, tag=f"qT{i}", name=f"qT{i}") for i in range(4)]
            kT = pp.tile([P, T], BF16, tag="kT")
            vT = pp.tile([P, T], BF16, tag="vT")
            # vext[kv * NTK + j] = [V_kv tile j | ones]  [128, 65]
            vext = [pp.tile([P, HD + 1], BF16, tag=f"ve{i}", name=f"ve{i}") for i in range(2 * NTK)]
            attnT = [pp.tile([P, T], BF16, tag=f"at{i}", name=f"at{i}") for i in range(4)]
            # dd[0] / dd[64] hold the pair's raw softmax denominators; the
            # middle rows stay zero (multiplied by ones2's zero rows)
            dd = pp.tile([HD + 1, TQ], BF16, tag="dd")
            nc.vector.memset(dd[:], 0.0)

            xts = [xw.tile([P, T], BF16, tag=f"x{j}", name=f"x{j}") for j in range(8)]
            wq = [xw.tile([P, QCOLS], BF16, tag=f"wq{j}", name=f"wq{j}") for j in range(8)]
            wk = [xw.tile([P, 128], BF16, tag=f"wk{j}", name=f"wk{j}") for j in range(8)]
            wv = [xw.tile([P, 128], BF16, tag=f"wv{j}", name=f"wv{j}") for j in range(8)]
            wo = [xw.tile([P, C], BF16, tag=f"wo{p}", name=f"wo{p}") for p in range(4)]

            # wq + the first x column-chunk first so Q-proj t=0 can start
            # early; alternate between the two HWDGE queues (SP / ACT) so the
            # ramp isn't serialized on one issue engine. ACT itself is idle
            # until attention starts, so borrowing its queue is free here.
            def dma_in(i, out, in_):
                eng = nc.sync if i % 2 == 0 else nc.scalar
                eng.dma_start(out=out, in_=in_)

            for j in range(8):
                dma_in(j, wq[j][:], wqT[P * j:P * (j + 1), :])
                dma_in(j + 1, xts[j][:, 0:TQ], xT[P * j:P * (j + 1), 0:TQ])
            for j in range(8):
                dma_in(j, wk[j][:], wkT[P * j:P * (j + 1), :])
                dma_in(j + 1, wv[j][:], wvT[P * j:P * (j + 1), :])
            for t in range(1, NTQ):
                for j in range(8):
                    # later chunks stay on the SP queue: the ACT queue must be
                    # clear before the first exps start
                    eng = nc.scalar if t == 1 and j % 2 == 0 else nc.sync
                    eng.dma_start(
                        out=xts[j][:, TQ * t:TQ * (t + 1)],
                        in_=xT[P * j:P * (j + 1), TQ * t:TQ * (t + 1)],
                    )
            for p in range(4):
                dma_in(p, wo[p][:], woT[P * p:P * (p + 1), :])

            # Deferred-emission queue: normalize tails and out-proj blocks
            # are emitted one pair-loop later so their serial DVE/PE chains
            # overlap the next pair's dense matmul work instead of
            # head-of-line blocking the PE.
            pending = []

            def norm_tail(p, cs, oA, oB, last=False):
                def emit():
                    # stack both raw denominator rows into dd, broadcast A
                    # onto partitions 0-63 / B onto 64-127 with one K=65
                    # matmul, reciprocal on all 128 lanes, then scale the PV
                    # outputs into attnT (all-bf16 muls hit the DVE fast path).
                    # For the last pair, oA/oB are the PSUM accumulators
                    # themselves (no staging copies — the ot slots are not
                    # needed again, and bc comes from the py tag to avoid
                    # deadlocking on the held ot slots).
                    nc.vector.tensor_copy(dd[0:1, :], oA[64:65, :])
                    nc.vector.tensor_copy(dd[64:65, :], oB[64:65, :])
                    bcpool = pyps if last else otps
                    bc = bcpool.tile([P, TQ], FP32, tag="py" if last else "ot",
                                     name="bc")
                    nc.tensor.matmul(bc[:], ones2[:], dd[:], start=True, stop=True)
                    rec = nrm.tile([P, TQ], FP32, tag="rec", name="rec")
                    nc.vector.reciprocal_approx_fast(rec[:], bc[:])
                    if last:
                        # PSUM operand + one fp32 SBUF operand: no bf16
                        # staging needed, and the SBUF same-start-partition
                        # rule sees only one SBUF input per mul
                        nc.vector.tensor_mul(attnT[p][0:64, cs], oA[0:64, :], rec[0:64, :])
                        nc.vector.tensor_mul(attnT[p][64:128, cs], oB[0:64, :], rec[64:128, :])
                        return
                    # split to two partition-0-based tiles: TensorTensor needs
                    # both SBUF inputs at the same start partition (copies may
                    # shift partitions, 2-input ALU ops may not)
                    recbA = nrm.tile([64, TQ], BF16, tag="recbA", name="recbA")
                    recbB = nrm.tile([64, TQ], BF16, tag="recbB", name="recbB")
                    nc.vector.tensor_copy(recbA[:], rec[0:64, :])
                    nc.vector.tensor_copy(recbB[:], rec[64:128, :])
                    nc.vector.tensor_mul(attnT[p][0:64, cs], oA[0:64, :], recbA[:])
                    nc.vector.tensor_mul(attnT[p][64:128, cs], oB[0:64, :], recbB[:])
                return emit

            def out_proj(c):
                def emit():
                    for tt in range(4):
                        t = 4 * c + tt
                        for co in range(2):
                            ps = pyps.tile([P, TQ], FP32, tag="py", name="yps")
                            for p in range(4):
                                nc.tensor.matmul(
                                    ps[:],
                                    attnT[p][:, P * t:P * (t + 1)],
                                    wo[p][:, TQ * co:TQ * (co + 1)],
                                    start=(p == 0), stop=(p == 3),
                                )
                            yt = ysb.tile([P, TQ], FP32, tag="yt")
                            nc.vector.tensor_copy(yt[:], ps[:])
                            nc.sync.dma_start(
                                out=y[P * t:P * (t + 1), TQ * co:TQ * (co + 1)],
                                in_=yt[:],
                            )
                return emit

            for t in range(NTQ):
                ts = slice(TQ * t, TQ * (t + 1))
                # ---- projections for token block t.
                if t == 0:
                    # Block 0 is DMA-arrival-bound: (wq[k], x0[k]) pairs land
                    # every ~790ns. Consume each arrival completely — all four
                    # f-groups' k-th matmuls (~850ns of PE work) — by running
                    # four concurrent PSUM accumulations. py has 2 slots; the
                    # attention 'ot' slots are still unused this early and are
                    # borrowed for the other two (same bank footprint,
                    # released at the copies well before attention starts).
                    qps = [
                        pyps.tile([P, TQ], FP32, tag="py", name="qp0"),
                        pyps.tile([P, TQ], FP32, tag="py", name="qp1"),
                        otps.tile([P, TQ], FP32, tag="ot", name="qp2"),
                        otps.tile([P, TQ], FP32, tag="ot", name="qp3"),
                    ]
                    for k in range(8):
                        for f in range(4):
                            nc.tensor.matmul(
                                qps[f][:], wq[k][:, P * f:P * (f + 1)],
                                xts[k][:, ts],
                                start=(k == 0), stop=(k == 7),
                            )
                    for f in range(4):
                        nc.vector.tensor_copy(qTs[f][:, ts], qps[f][:])
                else:
                    for f in range(4):
                        ps = pyps.tile([P, TQ], FP32, tag="py", name="qps")
                        for k in range(8):
                            nc.tensor.matmul(
                                ps[:], wq[k][:, P * f:P * (f + 1)], xts[k][:, ts],
                                start=(k == 0), stop=(k == 7),
                            )
                        nc.vector.tensor_copy(qTs[f][:, ts], ps[:])
                ps = pyps.tile([P, TQ], FP32, tag="py", name="kps")
                for k in range(8):
                    nc.tensor.matmul(
                        ps[:], wk[k][:], xts[k][:, ts],
                        start=(k == 0), stop=(k == 7),
                    )
                nc.vector.tensor_copy(kT[:, ts], ps[:])
                ps = pyps.tile([P, TQ], FP32, tag="py", name="vps")
                for k in range(8):
                    nc.tensor.matmul(
                        ps[:], wv[k][:], xts[k][:, ts],
                        start=(k == 0), stop=(k == 7),
                    )
                nc.vector.tensor_copy(vT[:, ts], ps[:])
                # V to token-major via PE transpose: [64, 128] -> [128, 64]
                for kv in range(2):
                    for jj in range(4):
                        j = 4 * t + jj
                        tp = pyps.tile([P, HD], BF16, tag="py", name="vtp")
                        nc.tensor.transpose(
                            tp[:],
                            vT[64 * kv:64 * kv + 64, P * j:P * (j + 1)],
                            ident2[64 * kv:64 * kv + 64, :],
                        )
                        ve = vext[kv * NTK + j]
                        nc.vector.tensor_copy(ve[:, 0:HD], tp[:])
                        nc.gpsimd.memset(ve[:, HD:HD + 1], 1.0)

                # ---- attention for query block c = t
                c = t
                cs = ts
                for p in range(4):
                    if c == NTQ - 1 and p == 3:
                        # flush queued normalize chains (DVE-bound, no PE
                        # cost) before the final pair's j-loop so they hide
                        # under its dense matmul work; out-proj thunks stay
                        # queued — their PE work fills this j-loop's
                        # ACT-gated bubbles
                        keep = []
                        for kind, thunk in pending:
                            if kind == "norm":
                                thunk()
                            else:
                                keep.append((kind, thunk))
                        pending[:] = keep
                    outA = otps.tile([HD + 1, TQ], FP32, tag="ot", name="otA")
                    outB = otps.tile([HD + 1, TQ], FP32, tag="ot", name="otB")
                    jmax = 4 * c + 3
                    for j in range(jmax + 1):
                        # scores for the pair land in one 2-bank tile:
                        # [:, 0, :] head A, [:, 1, :] head B
                        s = scps.tile([P, 2, TQ], FP32, tag="s")
                        nc.tensor.matmul(
                            s[:, 0, :],
                            kT[0:64, P * j:P * (j + 1)],
                            qTs[p][0:64, cs],
                            start=True, stop=True, tile_position=(0, 0),
                        )
                        nc.tensor.matmul(
                            s[:, 1, :],
                            kT[64:128, P * j:P * (j + 1)],
                            qTs[p][64:128, cs],
                            start=True, stop=True, tile_position=(64, 0),
                        )
                        pt = ptsb.tile([P, 2, TQ], BF16, tag="pt", name="pt")
                        r = j - 4 * c
                        # diagonal blocks (r >= 0): columns below 128r are
                        # fully masked — skip exp/mask/PV on them entirely
                        lo = 128 * r if r > 0 else 0
                        nc.scalar.activation(
                            pt[:, :, lo:TQ], s[:, :, lo:TQ], AF.Exp, scale=0.125
                        )
                        if r >= 0:
                            # only the 128-wide boundary block straddles the
                            # causal edge: keep where x >= tk in it
                            for h in range(2):
                                nc.gpsimd.affine_select(
                                    out=pt[:, h, lo:lo + P],
                                    in_=pt[:, h, lo:lo + P],
                                    compare_op=ALU.is_ge,
                                    fill=0.0,
                                    base=0,
                                    pattern=[[1, P]],
                                    channel_multiplier=-1,
                                )
                        nc.tensor.matmul(
                            outA[:, lo:TQ], vext[j][:], pt[:, 0, lo:TQ],
                            start=(j == 0), stop=(j == jmax),
                        )
                        nc.tensor.matmul(
                            outB[:, lo:TQ], vext[NTK + j][:], pt[:, 1, lo:TQ],
                            start=(j == 0), stop=(j == jmax),
                        )

                    # PSUM -> SBUF copies emitted immediately so the ot slots
                    # recycle fast; the rest of the normalize is deferred one
                    # pair-loop
                    if c == NTQ - 1 and p == 3:
                        # last pair: normalize straight from the PSUM
                        # accumulators, inline — nothing left to hide behind
                        norm_tail(p, cs, outA, outB, last=True)()
                    else:
                        oA = nrm.tile([HD + 1, TQ], BF16, tag="oA", name="oA")
                        oB = nrm.tile([HD + 1, TQ], BF16, tag="oB", name="oB")
                        nc.vector.tensor_copy(oA[:], outA[:])
                        nc.vector.tensor_copy(oB[:], outB[:])
                        npop = 2 if len(pending) >= 3 else 1
                        for _ in range(min(npop, len(pending))):
                            pending.pop(0)[1]()
                        pending.append(("norm", norm_tail(p, cs, oA, oB)))
                pending.append(("proj", out_proj(c)))
            for _, thunk in pending:
                thunk()

    nc.compile()
    return nc


def get_program():
    global _PROG
    if _PROG is None:
        _PROG = _build_program()
    return _PROG


def make_in_maps(x, Wq, Wk, Wv, Wo):
    """Build the per-core input dicts (host-side sharding + layout prep)."""
    import ml_dtypes
    BF = ml_dtypes.bfloat16
    x = np.asarray(x, np.float32)
    Wq = np.asarray(Wq, np.float32)
    Wk = np.asarray(Wk, np.float32)
    Wv = np.asarray(Wv, np.float32)
    Wo = np.asarray(Wo, np.float32)
    in_maps = []
    for core in range(8):
        b, half = core // 2, core % 2
        h0 = 8 * half
        kv0 = 2 * half
        # pair-permuted local head order: [h0, h0+4, h0+1, h0+5, ...]
        heads = []
        for p in range(4):
            heads += [h0 + p, h0 + p + 4]
        qrows = np.concatenate([Wq[h * HD:(h + 1) * HD] for h in heads], 0)  # [512, C]
        wocols = np.concatenate([Wo[:, h * HD:(h + 1) * HD] for h in heads], 1)  # [C, 512]
        in_maps.append({
            "xT": np.ascontiguousarray(x[b].T.astype(BF)),
            "wqT": np.ascontiguousarray(qrows.T.astype(BF)),
            "wkT": np.ascontiguousarray(Wk[kv0 * HD:(kv0 + 2) * HD].T.astype(BF)),
            "wvT": np.ascontiguousarray(Wv[kv0 * HD:(kv0 + 2) * HD].T.astype(BF)),
            "woT": np.ascontiguousarray(wocols.T.astype(BF)),
        })
    return in_maps


def run_on_hw(in_maps, trace=False, **kw):
    from concourse.bass_utils import run_bass_kernel_spmd
    nc = get_program()
    return run_bass_kernel_spmd(nc, in_maps, list(range(8)), trace=trace, **kw)


def kernel(**inputs):
    in_maps = make_in_maps(
        inputs["x"], inputs["Wq"], inputs["Wk"], inputs["Wv"], inputs["Wo"]
    )
    res = run_on_hw(in_maps)
    out = np.empty((4, T, C), np.float32)
    for b in range(4):
        out[b] = res.results[2 * b]["y"] + res.results[2 * b + 1]["y"]
    return out
